# revision 21
# baseline (speedup 1.0000x reference)
"""MoE location-expert router kernel for Trainium2 (8 NeuronCores).

Problem: out[i] = W[ptr[i] % 8] @ x[i] + b[ptr[i] % 8]
  x  [4096, 1024] f32, W [8, 32000, 1024] f32, b [8, 32000] f32 (zeros)
  out [4096, 32000] f32

Strategy (vocab / tensor-parallel sharding, W-stationary, mixed
fp16 + fp8-DoubleRow precision):
  - Host routes tokens: stable-sort by expert (NO padding).
  - Each of the 8 cores owns a 4000-wide slice of the vocab dim of ALL
    8 experts -> identical SPMD program on every core, perfectly load
    balanced regardless of the routing distribution.
  - Contraction split: channels 0:768 (6 K-chunks of 128) run fp16;
    channels 768:1024 run as ONE fp8e4 DoubleRow pass (2 K-slices
    packed per PE cell, 2 MACs/cell/cycle).  Both accumulate into the
    same PSUM bank: the fp8 operands carry compensating scales
    (x*2^-3, W*2^3) so the product needs no drain-time rescale.
    Measured rel_l2 vs f64 reference: 1.62e-2 (gate 2e-2); the fp16
    baseline was 3.2e-4.  This trades 7/8 of the matmul stream time
    and 7/8 of the instruction count for 8/8.
  - Vocab tiles are 128 wide (FWL needs NumWeights==128); tile 31
    overlaps tile 30 (v 3872:4000) so 32x128 covers the 4000 slice;
    the 96 duplicate rows are dropped on the host.
  - Per core, per expert, per 128-wide vocab tile: tokens stream in
    2+ groups of <=512 (PSUM bank limit); groups alternate PSUM banks.
  - Output is produced vocab-major [4096, 4096] per core; the host
    transposes + scatters back to [4096, 32000] (host time is free).
  - W fp16 loads in 512-wide chunks (1024B DRAM runs); W fp8 loads in
    1024-wide chunks (1KB runs).  PSUM tiles are always a full 2KB
    bank.  Three DMA rings: W16 on sync, x/w8 on scalar, outs on
    gpsimd.
"""

import os

import numpy as np
import ml_dtypes

import concourse.bacc as bacc
import concourse.bass as bass
import concourse.mybir as mybir
import concourse.tile as tile
from concourse.bass_utils import run_bass_kernel_spmd

E = 8          # experts
D = 1024       # d_model
V = 32000      # vocab
B = 4096       # tokens
NCORES = 8
VS = V // NCORES       # vocab slice per core (4000)
KT = 128               # contraction tile (partition dim)
KC16 = 6               # fp16 K-chunks (channels 0:768)
D16 = KC16 * KT        # 768
MT = 128               # vocab tile (FWL requires 128 weight columns)
NVT = 32               # vocab tiles per core; tile 31 overlaps tile 30
VPAD = NVT * MT        # padded vocab per core (4096)
WSUB = 4               # vocab tiles per fp16 W DMA chunk (512 wide)
W8CH = 1024            # fp8 W DMA chunk width (1KB runs)
GMAX = 512             # moving-group cap (PSUM bank = 512 f32)
SX = 2.0 ** -3         # fp8 x scale
SW = 2.0 ** 3          # fp8 W scale (SX*SW == 1 -> direct accumulation)
F8 = ml_dtypes.float8_e4m3

# host-side column gather: padded col j -> source vocab col in the 4000
# slice (tiles 0..30 identity, tile 31 = 3872:4000)
_VCOLS = np.concatenate([np.arange(31 * MT), np.arange(3872, 4000)])
# inverse: vocab col v in the 4000 slice -> row in the padded output
_VSEL = np.concatenate([np.arange(3968), np.arange(4064, 4096)])

_program_cache = {}


def _token_groups(c):
    """Split c tokens into ceil(c/512) nearly-equal groups (each >=245
    so the next LDWEIGHTS hides under the moving stream)."""
    if c == 0:
        return []
    ng = -(-c // GMAX)
    base = c // ng
    rem = c % ng
    sizes = [base + (1 if i < rem else 0) for i in range(ng)]
    offs = np.cumsum([0] + sizes[:-1]).tolist()
    return list(zip(offs, sizes))


def _pad16(n):
    return (n + 15) // 16 * 16


def _build_program(counts):
    """Trace the SPMD Tile program for the given per-expert counts."""
    io_dt = mybir.dt.float16
    out_dt = mybir.dt.float16
    f8_dt = mybir.dt.float8e4

    nc = bacc.Bacc("TRN2", target_bir_lowering=False, debug=False,
                   enable_asserts=False, num_devices=NCORES)

    xT = nc.dram_tensor("xT", [D16, B], io_dt, kind="ExternalInput").ap()
    x8 = nc.dram_tensor("x8", [KT, 2, B], f8_dt, kind="ExternalInput").ap()
    wT = nc.dram_tensor("wT", [E, D16, VPAD], io_dt,
                        kind="ExternalInput").ap()
    w8 = nc.dram_tensor("w8", [E, KT, 2, VPAD], f8_dt,
                        kind="ExternalInput").ap()
    out = nc.dram_tensor("out", [VPAD, B], out_dt, kind="ExternalOutput").ap()

    # [ (kc p) m -> p kc m ] views for K-chunked loads
    xT_r = xT.rearrange("(kc p) m -> p kc m", p=KT)

    with tile.TileContext(nc) as tc:
        with (
            tc.tile_pool(name="xp", bufs=2) as xpool,
            tc.tile_pool(name="x8p", bufs=2) as x8pool,
            tc.tile_pool(name="wp", bufs=8) as wpool,
            tc.tile_pool(name="w8p", bufs=2) as w8pool,
            tc.tile_pool(name="op", bufs=8) as opool,
            tc.tile_pool(name="ps", bufs=8, space="PSUM") as pspool,
        ):
            offs = np.cumsum([0] + [int(c) for c in counts[:-1]]).tolist()
            xts, x8ts, w8ts = {}, {}, {}

            def issue_x(e):
                """Allocate + DMA expert e's x (fp16 + fp8) and fp8 W."""
                c = int(counts[e])
                off = offs[e]
                xe = xpool.tile([KT, KC16, c], io_dt, tag="x")
                xe8 = x8pool.tile([KT, 2, _pad16(c)], f8_dt, tag="x8")
                w8t = w8pool.tile([KT, 2, VPAD], f8_dt, tag="w8")
                if e == 0:
                    # expert 0 gates kernel start: kc0 of the first token
                    # group alone first so the first matmul fires ASAP;
                    # fp8 x + fp8 W chunks on the (idle at start) out ring
                    n0 = _token_groups(c)[0][1]
                    nc.scalar.dma_start(out=xe[:, :1, :n0],
                                        in_=xT_r[:, :1, off:off + n0])
                    nc.scalar.dma_start(out=xe[:, :1, n0:],
                                        in_=xT_r[:, :1, off + n0:off + c])
                    nc.scalar.dma_start(out=xe[:, 1:3, :],
                                        in_=xT_r[:, 1:3, off:off + c])
                    nc.scalar.dma_start(out=xe[:, 3:, :],
                                        in_=xT_r[:, 3:, off:off + c])
                    nc.gpsimd.dma_start(out=xe8[:, :, :c],
                                        in_=x8[:, :, off:off + c])
                    for q in range(VPAD // W8CH):
                        nc.gpsimd.dma_start(
                            out=w8t[:, :, q * W8CH:(q + 1) * W8CH],
                            in_=w8[e][:, :, q * W8CH:(q + 1) * W8CH],
                        )
                else:
                    nc.scalar.dma_start(
                        out=xe[:, :KC16 // 2, :],
                        in_=xT_r[:, :KC16 // 2, off:off + c],
                    )
                    nc.scalar.dma_start(
                        out=xe[:, KC16 // 2:, :],
                        in_=xT_r[:, KC16 // 2:, off:off + c],
                    )
                    nc.scalar.dma_start(
                        out=xe8[:, :, :c],
                        in_=x8[:, :, off:off + c],
                    )
                    for q in range(VPAD // (2 * W8CH)):
                        nc.scalar.dma_start(
                            out=w8t[:, :, q * 2 * W8CH:(q + 1) * 2 * W8CH],
                            in_=w8[e][:, :, q * 2 * W8CH:(q + 1) * 2 * W8CH],
                        )
                xts[e], x8ts[e], w8ts[e] = xe, xe8, w8t

            first = True
            for e in range(E):
                c = int(counts[e])
                if c == 0:
                    continue
                off = offs[e]
                groups = _token_groups(c)
                issue_x(e)
                xe, xe8, w8t = xts.pop(e), x8ts.pop(e), w8ts.pop(e)
                wT_e = wT[e].rearrange("(kc p) v -> p kc v", p=KT)
                for wchunk in range(NVT // WSUB):
                    # 512-wide W chunk: contiguous 1024B DRAM runs
                    wt = wpool.tile([KT, KC16, WSUB * MT], io_dt, tag="w")
                    if first:
                        # kc-split, first vtile first, so the very first
                        # matmul only waits on a 32KB load
                        nc.sync.dma_start(
                            out=wt[:, :1, :MT],
                            in_=wT_e[:, :1, :MT],
                        )
                        nc.sync.dma_start(
                            out=wt[:, :1, MT:],
                            in_=wT_e[:, :1, MT:WSUB * MT],
                        )
                        nc.sync.dma_start(
                            out=wt[:, 1:3, :],
                            in_=wT_e[:, 1:3, :WSUB * MT],
                        )
                        nc.sync.dma_start(
                            out=wt[:, 3:, :],
                            in_=wT_e[:, 3:, :WSUB * MT],
                        )
                        first = False
                    else:
                        nc.sync.dma_start(
                            out=wt[:, :, :],
                            in_=wT_e[:, :, wchunk * WSUB * MT:
                                     (wchunk + 1) * WSUB * MT],
                        )
                    for s in range(WSUB):
                        vt = wchunk * WSUB + s
                        # full-bank psum tiles: two tiles must never share
                        # a 2KB bank or bank-aware dep tracking serializes
                        # matmuls against the other tile's drain
                        pts = [pspool.tile([MT, GMAX], mybir.dt.float32,
                                           tag="ps", name=f"ps{g}")
                               for g, (_, n) in enumerate(groups)]
                        for kc in range(KC16):
                            for g, (g0, n) in enumerate(groups):
                                nc.tensor.matmul(
                                    pts[g][:, :n],
                                    wt[:, kc, s * MT:(s + 1) * MT],
                                    xe[:, kc, g0:g0 + n],
                                    start=(kc == 0), stop=False,
                                )
                        # fp8 DoubleRow pass: channels 768:1024, two
                        # 128-deep K-slices packed, accumulates into the
                        # same bank (SX*SW==1)
                        for g, (g0, n) in enumerate(groups):
                            nc.tensor.matmul(
                                pts[g][:, :n],
                                w8t[:, :, vt * MT:(vt + 1) * MT],
                                xe8[:, :, g0:g0 + n],
                                start=False, stop=True,
                                perf_mode=mybir.MatmulPerfMode.DoubleRow,
                            )
                        # one merged out tile + one DMA per (e, vt); the
                        # last expert alternates outs onto the (by then
                        # idle) W ring to halve the end-of-kernel
                        # out-queue drain
                        ot = opool.tile([MT, c], out_dt, tag="o")
                        for g, (g0, n) in enumerate(groups):
                            if (vt + g) % 2 == 0:
                                nc.vector.tensor_copy(ot[:, g0:g0 + n],
                                                      pts[g][:, :n])
                            else:
                                nc.scalar.copy(ot[:, g0:g0 + n],
                                               pts[g][:, :n])
                        if e == E - 1:
                            # spread the last expert's outs over four
                            # rings: the W ring is idle by then and the
                            # copy engines' queues flush in parallel,
                            # shrinking the end-of-kernel out drain
                            oring = (nc.gpsimd, nc.sync,
                                     nc.scalar)[vt % 3]
                        else:
                            oring = nc.gpsimd
                        oring.dma_start(
                            out=out[vt * MT:(vt + 1) * MT, off:off + c],
                            in_=ot[:, :],
                        )
    nc.compile()
    return nc


def _get_program(counts):
    key = tuple(int(c) for c in counts)
    if key not in _program_cache:
        _program_cache[key] = _build_program(key)
    return _program_cache[key]


def _prepare(x, pointer_addresses, W):
    idx = (np.asarray(pointer_addresses).astype(np.int64) % E).astype(np.int32)
    counts = np.bincount(idx, minlength=E)
    order = np.argsort(idx, kind="stable")
    nc = _get_program(counts)

    x = np.asarray(x, dtype=np.float32)
    xs = x[order]                                   # [B, D] sorted
    xT = np.ascontiguousarray(xs[:, :D16].astype(np.float16).T)  # [768, B]
    # fp8 pair: channels 768:1024, scaled by SX, laid out [128, 2, B]
    x8f = (xs[:, D16:] * SX).astype(np.float32).astype(F8)       # [B, 256]
    x8 = np.ascontiguousarray(
        x8f.T.reshape(2, KT, B).transpose(1, 0, 2))              # [128, 2, B]

    W = np.asarray(W)
    wts, w8s = [], []
    for c in range(NCORES):
        Wc = W[:, c * VS:(c + 1) * VS, :][:, _VCOLS, :]   # [E, VPAD, D]
        WTc = np.ascontiguousarray(
            Wc[:, :, :D16].transpose(0, 2, 1)).astype(np.float16)
        wts.append(WTc)                                   # [E, 768, VPAD]
        w8f = (Wc[:, :, D16:] * SW).astype(np.float32).astype(F8)
        w8c = np.ascontiguousarray(
            w8f.transpose(0, 2, 1)                        # [E, 256, VPAD]
            .reshape(E, 2, KT, VPAD).transpose(0, 2, 1, 3))  # [E,128,2,VPAD]
        w8s.append(w8c)
    return idx, order, nc, xT, x8, wts, w8s


def _run(x, pointer_addresses, W, b, trace=False):
    idx, order, nc, xT, x8, wts, w8s = _prepare(x, pointer_addresses, W)
    in_maps = [{"xT": xT, "x8": x8, "wT": wts[c], "w8": w8s[c]}
               for c in range(NCORES)]
    kw = {}
    if trace:
        kw = dict(trace=True, trace_cores=[0])
    res = run_bass_kernel_spmd(nc, in_maps, list(range(NCORES)), **kw)

    out = np.empty((B, V), dtype=np.float32)
    for c in range(NCORES):
        # res [VPAD, B] fp16 vocab-major; drop the 96 overlap rows and
        # transpose to [B, VS]
        out[order, c * VS:(c + 1) * VS] = res.results[c]["out"][_VSEL].T

    b = np.asarray(b)
    if b.any():
        for e in range(E):
            out[idx == e] += b[e].astype(np.float32)
    return out, res


def kernel(x, pointer_addresses, W, b):
    out, _ = _run(x, pointer_addresses, W, b, trace=False)
    return out
